# revision 42
# baseline (speedup 1.0000x reference)
"""Trainium2 Bass kernel for nn_AttentionBlock (B=8, C=512, H=W=32, 8 heads).

Sharding: data-parallel over batch — core b computes batch image b end-to-end
(attention is independent per (batch, head); weights replicated to all cores).

Per-core pipeline (x viewed as (C=512, S=1024), channels-on-partition):
  P1a: q,k = Wqk^T.T @ x (bf16 in, f32 PSUM->SBUF), channel order arranged on
       host so each 128-row m-tile is one head-PAIR of q or k.
  P1b: vT = x.T @ (16*Wv) as fp8 DoubleRow matmuls (2 k-tiles per instr,
       0.5 cycles/row).  Evicted to SBUF fp8 with a 16-valued column per head
       (head stride 66 keeps the DoubleRow weight AP 16B-aligned); the
       denominator row scales by the same 16, so softmax is exact.
  P2 : scoresT[t,s] per (head, j) into a double-buffered (128,1024) PSUM slot;
       emission runs one j-pair AHEAD of P3 so the PE always has the next
       scores ready before the ACT needs them.
  exp: ACT exp(s/8 - 2) PSUM->SBUF fp8e4, one instr per j (the ACT is the
       bottleneck engine at ~68us busy; everything else hides under it).
       The -2 bias (shipped as an extra weights-DMA column) is
       softmax-invariant and keeps exp outputs inside fp8e4 range.
  P3 : attn@v as fp8 DoubleRow matmuls — each instruction contracts BOTH
       j-tiles of a pair (2x128 rows) at 0.5 cycles/row.  Row 64 of the
       65-row output = 16*denominator.
  norm: DVE reciprocal -> ones outer-product PE broadcast -> DVE multiply,
       flushed in two stages one/two units after the head so the PE FIFO
       never parks on a reciprocal-dependent broadcast.
  head 7 runs as two n-half passes: its n0 attention output, normalize and
       P4 n0-half tail (full k-loop matmuls + adds + y DMAs) complete while
       the n1 exps still stream; the endgame holds only the n1 norm chain,
       pair-3 k-instrs (k0-2 pre-accumulated into the freed score banks),
       four adds, and a y DMA fan-out whose ~1.3us launch latencies
       parallelize across the three DGE queues.

PE warmup matmuls run during the initial DMA so the p-state ramp completes
before real work.  Cross-engine waits are kept inside walrus's single-wait
instruction budget with cost-free standalone ldweights carriers (PE), tiny
scratch copies (DVE), and _strip_self_waits.
"""

import os
import sys

for _p in ("/opt/trn_rl_repo", "/root/.axon_site/_ro/trn_rl_repo"):
    if os.path.isdir(_p) and _p not in sys.path:
        sys.path.insert(0, _p)

from contextlib import ExitStack

import ml_dtypes
import numpy as np

import concourse.bass as bass
import concourse.tile as tile
from concourse import mybir
from concourse.bass_utils import run_bass_kernel_spmd

B, C, H, W = 8, 512, 32, 32
NH, D = 8, 64
S = H * W            # 1024 sequence positions
P = 128              # partitions
KT = C // P          # 4 contraction tiles over channels
NT = S // P          # 8 t-tiles
NPAIR = NH // 2      # 4 head pairs
HB = 66              # vta head block: 64 v cols + denom col + pad (16B stride)
F32 = mybir.dt.float32
BF16 = mybir.dt.bfloat16
FP8 = mybir.dt.float8e4
AF = mybir.ActivationFunctionType
ALU = mybir.AluOpType
PM = mybir.MatmulPerfMode

NWARM = int(os.environ.get("K_NWARM", "4"))
EXP_BIAS = -2.0      # exp(s/8 - 2): softmax-invariant, keeps fp8e4 in range
WVS = 16.0           # host scale on Wv; denom column = 16 compensates exactly


def _r(ap):
    """float32r view: fp32 bits at full-rate 1 cycle/row in the PE."""
    return ap.bitcast(mybir.dt.float32r)


def _install_drain_split():
    """walrus's CTRL_NO (drain) codegen accepts only a single semaphore wait,
    but Tile's kernel-tail drain aggregates one wait per live proc.  Split
    them across several serial drains (semantically identical: all complete
    before the closing all-engine barrier)."""
    if getattr(tile.TileContext, "_drain_split_installed", False):
        return
    from concourse.vector_clock import ScopedClock

    orig = tile.TileContext._drain_and_barrier

    def patched(self, tick_clock, wait_clock):
        nc = self.nc
        drain_inst = nc.sync.drain()
        wait_clock.add_sem_waits(
            drain_inst.ins, ScopedClock({None: tick_clock.global_clock})
        )
        si = drain_inst.ins.sync_info
        if si is not None and si.on_wait and len(si.on_wait) > 1:
            waits = list(si.on_wait)
            drain_inst.ins.sync_info = mybir.SyncInfo(
                on_wait=[waits[0]], on_update=list(si.on_update or [])
            )
            for w in waits[1:]:
                d2 = nc.sync.drain()
                d2.ins.sync_info = mybir.SyncInfo(on_wait=[w], on_update=[])

        nc.all_engine_barrier()
        assert self.sems is not None
        popped = nc._tile_sem_poison_stack.pop()
        assert popped is self._sem_poison
        nc.clear_and_free_semaphores(list(self.sems.allocated().values()))
        nc.all_engine_barrier()

    tile.TileContext._drain_and_barrier = patched
    tile.TileContext._drain_split_installed = True
    tile.TileContext._drain_and_barrier_orig = orig


def trace_kernel(ctx, tc, nc, xb, xq_d, xf_d, wb, wvq_d, wf, y):
    cst = ctx.enter_context(tc.tile_pool(name="cst", bufs=1))
    qkp = ctx.enter_context(tc.tile_pool(name="qkp", bufs=5))
    expp = ctx.enter_context(tc.tile_pool(name="expp", bufs=8))
    rdp = ctx.enter_context(tc.tile_pool(name="rdp", bufs=3))
    rbp = ctx.enter_context(tc.tile_pool(name="rbp", bufs=3))
    pa = ctx.enter_context(tc.tile_pool(name="pa", bufs=2, space="PSUM"))
    psc = ctx.enter_context(tc.tile_pool(name="psc", bufs=2, space="PSUM"))
    poa = ctx.enter_context(tc.tile_pool(name="poa", bufs=1, space="PSUM"))

    xt = cst.tile([P, KT, S], BF16)
    xq = cst.tile([P, KT, S], FP8)
    xf = cst.tile([P, KT, S], F32)
    wq = cst.tile([P, KT, 2 * C], BF16)
    wvq = cst.tile([P, KT, C], FP8)
    wo = cst.tile([P, KT, C], F32)
    bob = cst.tile([P, KT, 2], F32)
    onesf = cst.tile([1, D], F32)
    wrm = cst.tile([P, 512], BF16)
    vta = cst.tile([P, NT, NH * HB], FP8)
    res = cst.tile([P, NPAIR, S], F32)
    ybig = cst.tile([P, KT, S], F32)
    scr = cst.tile([1, 256], F32)

    scr_i = [0]

    def dve_sync(*aps):
        # DVE wait-carrier: absorb one cross-engine wait per tiny copy.
        # Disjoint scratch columns avoid WAW self-waits between carriers.
        for ap in aps:
            n = ap.free_size()
            o = (scr_i[0] % 30) * 8
            scr_i[0] += 1
            nc.vector.tensor_copy(scr[0:1, o:o + n], ap)

    def pe_ld(ap):
        # PE wait-carrier: a standalone ldweights is cost-free and absorbs
        # one cross-engine wait into PE program order (f32 views bitcast to
        # bf16 — ldweights rejects 4-byte dtypes).
        if ap.dtype in (F32, mybir.dt.float32r):
            ap = ap.bitcast(BF16)
        nc.tensor.ldweights(ap)

    def pe_ld_dve(ap):
        # pe_ld needs a partition-0-aligned source; for DVE-written regions
        # on other partitions, bounce the tick through a scratch column.
        n = ap.free_size()
        o = (scr_i[0] % 30) * 8
        scr_i[0] += 1
        nc.vector.tensor_copy(scr[0:1, o:o + n], ap)
        pe_ld(scr[0:1, o:o + n])

    # ---- warmup + on-chip consts (no DMA deps) ----
    nc.vector.memset(wrm[:, :], 0.125)
    # denominator column per head in the augmented vT (= WVS, matching the
    # host-side WVS scale on Wv)
    vones = vta.rearrange("p j (h e) -> p j h e", e=HB)[:, :, :, D:D + 1]
    nc.vector.memset(vones, WVS)
    # ones row for the 1/denom broadcast outer product (ACT absorbs the
    # DVE-memset waits here, acting as the ACT-side carrier)
    nc.scalar.activation(_r(onesf[:, :]), wrm[0:1, 0:D], AF.Exp, scale=0.0)
    for i in range(NWARM):
        wt = pa.tile([P, 512], F32, tag="sp", name=f"wrm{i}")
        nc.tensor.matmul(wt[:, :], wrm[:, 0:P], wrm[:, :],
                         start=True, stop=True, skip_group_check=True)

    # ---- DMAs: startup-critical chain on the sync (HWDGE) queue, bulk on
    # the gpsimd queue so the two streams proceed independently ----
    xbr = xb.rearrange("(k p) s -> p k s", p=P)
    xqr = xq_d.rearrange("(k p) s -> p k s", p=P)
    wbr = wb.rearrange("(k p) s -> p k s", p=P)
    wvr = wvq_d.rearrange("(k p) s -> p k s", p=P)
    wfr = wf.rearrange("(k p) s -> p k s", p=P)
    xfr = xf_d.rearrange("(k p) s -> p k s", p=P)
    nc.sync.dma_start(out=xt[:, :, 0:512], in_=xbr[:, :, 0:512])
    nc.sync.dma_start(out=wq[:, :, 0:256], in_=wbr[:, :, 0:256])
    nc.sync.dma_start(out=bob[:, :, :], in_=wfr[:, :, C:C + 2])
    nc.sync.dma_start(out=xt[:, :, 512:S], in_=xbr[:, :, 512:S])
    nc.sync.dma_start(out=xq[:, :, :], in_=xqr[:, :, :])
    nc.sync.dma_start(out=wvq[:, :, :], in_=wvr[:, :, :])
    nc.sync.dma_start(out=wq[:, :, 256:2 * C], in_=wbr[:, :, 256:2 * C])
    nc.sync.dma_start(out=_r(wo[:, :, :]), in_=_r(wfr[:, :, 0:C]))
    nc.sync.dma_start(out=xf[:, :, :], in_=xfr[:, :, :])
    # absorb the bias-column DMA tick into the ACT clock (exps read it)
    nc.scalar.copy(scr[0:1, 252:254], bob[0:1, 0, 0:2])

    qk = [None] * NPAIR

    def p1a_half(m, n):
        pair, isk = divmod(m, 2)
        if isk == 0 and n == 0:
            qk[pair] = qkp.tile([P, 2 * S], F32, tag="qk", name=f"qk{pair}")
        pe_ld(wq[0:1, 0, m * P:m * P + 2])
        pe_ld(xt[0:1, 0, n * 512:n * 512 + 2])
        acc = pa.tile([P, 512], F32, tag="sp", name=f"pacc{m}_{n}")
        for k in range(KT):
            nc.tensor.matmul(
                acc[:, :],
                wq[:, k, m * P:(m + 1) * P],
                xt[:, k, n * 512:(n + 1) * 512],
                start=(k == 0),
                stop=(k == KT - 1),
            )
        nc.vector.tensor_copy(
            _r(qk[pair][:, isk * S + n * 512: isk * S + (n + 1) * 512]),
            _r(acc[:, :]),
        )

    def p1b(j):
        # vT j-tile via fp8 DoubleRow: 2 instrs contract all 4 k-tiles
        pe_ld(wvq[0:1, 0, 0:2])
        pe_ld(xq[0:1, 0, 0:2])
        acc = pa.tile([P, 512], F32, tag="sp", name=f"vacc{j}")
        for kk in range(2):
            nc.tensor.matmul(
                acc[:, :],
                xq[:, 2 * kk:2 * kk + 2, j * P:(j + 1) * P],
                wvq[:, 2 * kk:2 * kk + 2, :],
                start=(kk == 0), stop=(kk == 1),
                perf_mode=PM.DoubleRow, skip_group_check=True,
            )
        nc.vector.tensor_copy(
            vta[:, j, :].rearrange("p (h e) -> p h e", e=HB)[:, :, 0:D],
            acc.rearrange("p (h d) -> p h d", h=NH),
        )

    # filler schedule: PE work slotted into the attention j-pair windows
    fillers = {
        (0, 0): [lambda: p1b(0), lambda: p1b(1), lambda: p1a_half(1, 1),
                 lambda: pe_ld(qk[0][0:1, 2 * S - 2:2 * S])],
        (0, 1): [lambda: p1b(2), lambda: p1b(3)],
        (0, 2): [lambda: p1b(4), lambda: p1b(5)],
        (0, 3): [lambda: p1b(6), lambda: p1b(7)],
    }
    half_order = [(m, n) for pr in (1, 2, 3) for n in (0, 1)
                  for m in (2 * pr, 2 * pr + 1)]
    half_slots = [(1, 0), (1, 0), (1, 1), (1, 2),
                  (1, 3), (2, 0), (2, 1), (2, 2),
                  (2, 3), (3, 0), (3, 1), (3, 2)]
    for (m, n), (h, jp) in zip(half_order, half_slots):
        fillers.setdefault((h, jp), []).append(lambda m=m, n=n: p1a_half(m, n))

    oa_box = [None]

    def norm_recip(h, n):
        oa = oa_box[0]
        # fold the P3 (PE) tick into the DVE clock so the later multiply
        # carries only its rb wait (walrus single-wait + race safety)
        dve_sync(oa[0:1, n * 512:n * 512 + 2])
        rd = rdp.tile([1, 512], F32, tag="rd", name=f"rd{h}_{n}")
        with nc.allow_low_precision(reason="softmax reciprocal"):
            nc.vector.reciprocal(_r(rd[:, :]),
                                 oa[D:D + 1, n * 512:(n + 1) * 512])
        return rd

    def norm_apply(h, n, rd):
        pair, hh = divmod(h, 2)
        oa = oa_box[0]
        rb = rbp.tile([D, 512], F32, tag="rb", name=f"rb{h}_{n}")
        bc = pa.tile([D, 512], F32, tag="sp", name=f"bc{h}_{n}")
        nc.tensor.matmul(
            bc[:, :], _r(onesf[:, :]), _r(rd[0:1, :]),
            start=True, stop=True, skip_group_check=True,
        )
        nc.vector.tensor_copy(rb[:, :], bc[:, :])
        nc.vector.tensor_mul(
            _r(res[D * hh:D * (hh + 1), pair, n * 512:(n + 1) * 512]),
            _r(oa[0:D, n * 512:(n + 1) * 512]), _r(rb[:, :]),
        )

    def norm_half(h, n):
        norm_apply(h, n, norm_recip(h, n))

    ets = {}

    def emit_p2exp(h, jp):
        pair, hh = divmod(h, 2)
        base = D * hh
        qkt = qk[pair]
        if hh == 0 and jp == 0:
            # fresh pair: fold the qk-eviction DVE tick into PE program order
            pe_ld(qkt[0:1, 2 * S - 2:2 * S])
        et = expp.tile([P, 2, S], FP8, tag="et", name=f"et{h}_{jp}")
        ets[(h, jp)] = et
        for jj in range(2):
            j = 2 * jp + jj
            sc = psc.tile([P, S], F32, tag="sc", name=f"sc{h}_{j}")
            for n in range(2):
                nc.tensor.matmul(
                    sc[:, n * 512:(n + 1) * 512],
                    _r(qkt[base:base + D, S + j * P: S + (j + 1) * P]),
                    _r(qkt[base:base + D, n * 512:(n + 1) * 512]),
                    start=True, stop=True, skip_group_check=True,
                )
            nc.scalar.activation(
                et[:, jj, :], sc[:, :], AF.Exp,
                scale=1.0 / np.sqrt(D), bias=bob[:, 0, 1:2],
            )

    def emit_p2exp7(n, jp):
        # head 7 runs as two n-half passes: its n0 attention output, norm and
        # P4 tail complete while the n1 exps still stream on the ACT
        qkt = qk[NPAIR - 1]
        et = expp.tile([P, 2, S], FP8, tag="et", name=f"et7_{n}_{jp}")
        ets[(7, n, jp)] = et
        for jj in range(2):
            j = 2 * jp + jj
            sc = psc.tile([P, S], F32, tag="sc", name=f"sc7_{n}_{j}")
            nc.tensor.matmul(
                sc[:, n * 512:(n + 1) * 512],
                _r(qkt[D:2 * D, S + j * P: S + (j + 1) * P]),
                _r(qkt[D:2 * D, n * 512:(n + 1) * 512]),
                start=True, stop=True, skip_group_check=True,
            )
            nc.scalar.activation(
                et[:, jj, n * 512:(n + 1) * 512],
                sc[:, n * 512:(n + 1) * 512], AF.Exp,
                scale=1.0 / np.sqrt(D), bias=bob[:, 0, 1:2],
            )

    def alloc_oa(h):
        oa_box[0] = poa.tile([D + 1, S], F32, tag="oa", name=f"oa{h}")
        if h > 0:
            # absorb the WAR on oa vs. the previous head's norm reads
            ppair, phh = divmod(h - 1, 2)
            pe_ld_dve(res[D * phh:D * phh + 1, ppair, 0:2])

    def emit_p3(h, jp):
        if jp == 0:
            alloc_oa(h)
            if h <= 1:
                pe_ld(vta[0:1, 2 * jp + 1, 0:2])
        elif h == 0:
            pe_ld(vta[0:1, 2 * jp + 1, 0:2])
        oa = oa_box[0]
        et = ets.pop((h, jp))
        for n in range(2):
            nc.tensor.matmul(
                oa[:, n * 512:(n + 1) * 512],
                vta[:, 2 * jp:2 * jp + 2, h * HB: h * HB + D + 1],
                et[:, :, n * 512:(n + 1) * 512],
                start=(jp == 0), stop=(jp == 3),
                perf_mode=PM.DoubleRow, skip_group_check=True,
            )

    def emit_p3_7(n, jp):
        if n == 0 and jp == 0:
            alloc_oa(7)
        oa = oa_box[0]
        et = ets.pop((7, n, jp))
        nc.tensor.matmul(
            oa[:, n * 512:(n + 1) * 512],
            vta[:, 2 * jp:2 * jp + 2, 7 * HB: 7 * HB + D + 1],
            et[:, :, n * 512:(n + 1) * 512],
            start=(jp == 0), stop=(jp == 3),
            perf_mode=PM.DoubleRow, skip_group_check=True,
        )

    yr = y.rearrange("(k p) s -> p k s", p=P)
    p4acc = {}

    def tail_mm(m, n):
        # P4 m-tile n-half: all four k-instrs (res pair 3 arrives last)
        acc = pa.tile([P, 512], F32, tag="sp", name=f"p4{m}_{n}")
        p4acc[(m, n)] = acc
        for k in range(KT):
            nc.tensor.matmul(
                acc[:, :],
                _r(wo[:, k, m * P:(m + 1) * P]),
                _r(res[:, k, n * 512:(n + 1) * 512]),
                start=(k == 0), stop=(k == KT - 1),
            )

    def tail_add(m, n, q):
        # y = acc + bo + x, then the y half-DMA on an HWDGE queue (SWDGE
        # desc-gen on the Pool engine is ~1.2us per DMA and would serialize
        # the tail)
        nc.vector.scalar_tensor_tensor(
            ybig[:, m, n * 512:(n + 1) * 512], p4acc.pop((m, n))[:, :],
            bob[:, m, 0:1], xf[:, m, n * 512:(n + 1) * 512],
            op0=ALU.add, op1=ALU.add,
        )
        q.dma_start(
            out=yr[:, m, n * 512:(n + 1) * 512],
            in_=ybig[:, m, n * 512:(n + 1) * 512],
        )

    # ---- schedule trace ----
    p1a_half(0, 0)
    p1a_half(1, 0)
    p1a_half(0, 1)
    pend_recip = [None]
    pend_rest = [None]

    def flush_recips():
        # stage 1 (one unit after the head ends): DVE reciprocals only —
        # emitting the PE broadcast here would stall the PE FIFO on them
        if pend_recip[0] is not None:
            h = pend_recip[0]
            pend_recip[0] = None
            pend_rest[0] = (h, norm_recip(h, 0), norm_recip(h, 1))

    def flush_rest():
        # stage 2 (next unit): broadcast + multiply; the recips are long done
        if pend_rest[0] is not None:
            h, rd0, rd1 = pend_rest[0]
            pend_rest[0] = None
            norm_apply(h, 0, rd0)
            norm_apply(h, 1, rd1)

    units = [(h, jp) for h in range(NH - 1) for jp in range(4)]
    emit_p2exp(0, 0)
    for i, (h, jp) in enumerate(units):
        # P2/exp run one j-pair ahead so the ACT never waits on the PE;
        # norms flush in two stages and P3(h,0) defers one unit so the PE
        # FIFO never parks on a reciprocal-dependent broadcast
        if i + 1 < len(units):
            emit_p2exp(*units[i + 1])
        else:
            emit_p2exp7(0, 0)
        flush_rest()
        for f in fillers.get((h, jp), ()):
            f()
        flush_recips()
        if jp == 0 and h > 0:
            pass            # P3(h,0) deferred to the jp==1 unit
        elif jp == 1 and h > 0:
            emit_p3(h, 0)
            emit_p3(h, 1)
        else:
            emit_p3(h, jp)
        if jp == 3:
            pend_recip[0] = h
    # head 7, pass n0
    for jp in range(4):
        if jp < 3:
            emit_p2exp7(0, jp + 1)
        else:
            emit_p2exp7(1, 0)
        flush_rest()
        for f in fillers.get((70, jp), ()):
            f()
        flush_recips()
        if jp == 0:
            pass            # P3_7(0,0) deferred to the jp==1 unit
        elif jp == 1:
            emit_p3_7(0, 0)
            emit_p3_7(0, 1)
        else:
            emit_p3_7(0, jp)
    # head 7, pass n1 — the whole n0-half P4 tail (matmuls, adds, y DMAs)
    # runs inside this pass, overlapped with the n1 exp stream
    rd70 = [None]
    for jp in range(4):
        if jp < 3:
            emit_p2exp7(1, jp + 1)
        if jp == 1:
            rd70[0] = norm_recip(7, 0)
        if jp == 2:
            norm_apply(7, 0, rd70[0])
            dve_sync(xf[0:1, 0, 0:2], bob[0:1, 0, 0:1])
            tail_mm(0, 0)
            tail_mm(1, 0)
        if jp == 3:
            emit_p3_7(1, jp)
            dve_sync(oa_box[0][0:1, 512:514])
            rd7 = rdp.tile([1, 512], F32, tag="rd", name="rd7_1")
            with nc.allow_low_precision(reason="softmax reciprocal"):
                nc.vector.reciprocal(_r(rd7[:, :]),
                                     oa_box[0][D:D + 1, 512:S])
            tail_add(0, 0, nc.sync)
            tail_mm(2, 0)
            tail_add(1, 0, nc.scalar)
            tail_mm(3, 0)
            tail_add(2, 0, nc.gpsimd)
            tail_add(3, 0, nc.sync)
        else:
            emit_p3_7(1, jp)
    # endgame: rb copy on the now-idle ACT; n1 P4 k0-2 pre-accumulate into
    # the freed score banks (they only need res pairs 0-2) so only the
    # k3-instr and final add trail the last normalize; y DMA launch
    # latencies (~1.3us each) parallelize across all three DGE queues
    rb = rbp.tile([D, 512], F32, tag="rbt", name="rb7_1")
    bc = pa.tile([D, 512], F32, tag="sp", name="bc7_1")
    nc.tensor.matmul(bc[:, :], _r(onesf[:, :]), _r(rd7[0:1, :]),
                     start=True, stop=True, skip_group_check=True)
    nc.scalar.copy(rb[:, :], bc[:, :])
    n1acc = []
    for i in range(2):
        a2 = psc.tile([P, S], F32, tag="sc", name=f"p4n1_{i}")
        n1acc.append(a2)
    for m in range(KT):
        acc = n1acc[m // 2][:, (m % 2) * 512:(m % 2) * 512 + 512]
        for k in range(KT - 1):
            nc.tensor.matmul(
                acc, _r(wo[:, k, m * P:(m + 1) * P]),
                _r(res[:, k, 512:S]),
                start=(k == 0), stop=False, skip_group_check=True,
            )
    nc.vector.tensor_mul(_r(res[D:2 * D, NPAIR - 1, 512:S]),
                         _r(oa_box[0][0:D, 512:S]), _r(rb[:, :]))
    yq = [nc.sync, nc.scalar, nc.gpsimd, nc.sync]
    for m in range(KT):
        acc = n1acc[m // 2][:, (m % 2) * 512:(m % 2) * 512 + 512]
        nc.tensor.matmul(
            acc, _r(wo[:, KT - 1, m * P:(m + 1) * P]),
            _r(res[:, KT - 1, 512:S]),
            start=False, stop=True, skip_group_check=True,
        )
        nc.vector.scalar_tensor_tensor(
            ybig[:, m, 512:S], acc, bob[:, m, 0:1], xf[:, m, 512:S],
            op0=ALU.add, op1=ALU.add,
        )
        yq[m].dma_start(out=yr[:, m, 512:S], in_=ybig[:, m, 512:S])


ENGINE_SEM_PREFIX = {
    "PE": "PE_",
    "Activation": "Activation_",
    "DVE": "DVE_",
    "Pool": "Pool_",
    "SP": "SP_",
}


def _strip_self_waits(nc):
    """Drop same-engine semaphore self-waits from multi-wait instructions.

    Engines execute and complete their own instructions in program order
    (PE matmuls are pc-monotone in start and end; ACT/DVE/Pool are strict
    FIFO with per-op drains), so a wait on the engine's own completion
    semaphore is redundant whenever the instruction carries another wait —
    and walrus's PE/ACT instruction structs only encode a single wait.
    """
    n = 0
    for inst in nc.inst_map.values():
        si = getattr(inst, "sync_info", None)
        if si is None or not si.on_wait or len(si.on_wait) <= 1:
            continue
        eng = str(getattr(inst, "engine", "")).split(".")[-1]
        if type(inst).__name__ == "InstDMACopy":
            keep = [w for w in si.on_wait if not w.ant_name.startswith("DMAHW")]
            if keep and len(keep) != len(si.on_wait):
                inst.sync_info = mybir.SyncInfo(
                    on_wait=keep, on_update=list(si.on_update or [])
                )
                n += 1
            continue
        pref = ENGINE_SEM_PREFIX.get(eng)
        if pref is None:
            continue
        keep = [w for w in si.on_wait if not w.ant_name.startswith(pref)]
        if len(keep) != len(si.on_wait) and keep:
            inst.sync_info = mybir.SyncInfo(
                on_wait=keep, on_update=list(si.on_update or [])
            )
            n += 1
    return n


def build_nc():
    _install_drain_split()
    nc = bass.Bass(trn_type="TRN2", debug=False, num_devices=8)

    xb_d = nc.dram_tensor("xb", [C, S], BF16, kind="ExternalInput")
    xq_d = nc.dram_tensor("xq", [C, S], FP8, kind="ExternalInput")
    xf_d = nc.dram_tensor("xf", [C, S], F32, kind="ExternalInput")
    wb_d = nc.dram_tensor("wb", [C, 2 * C], BF16, kind="ExternalInput")
    wvq_d = nc.dram_tensor("wvq", [C, C], FP8, kind="ExternalInput")
    wf_d = nc.dram_tensor("wf", [C, C + 2], F32, kind="ExternalInput")
    y_d = nc.dram_tensor("y", [C, S], F32, kind="ExternalOutput")
    with tile.TileContext(nc) as tc, ExitStack() as ctx:
        trace_kernel(ctx, tc, nc, xb_d.ap(), xq_d.ap(), xf_d.ap(), wb_d.ap(),
                     wvq_d.ap(), wf_d.ap(), y_d.ap())
    _strip_self_waits(nc)
    if not nc.is_finalized():
        nc.finalize()
    return nc


def host_inputs(x, Wqkv, Wo, bo):
    """Host-side reshard: per-core input dicts (weights replicated)."""
    x = np.ascontiguousarray(np.asarray(x, dtype=np.float32))
    Wqkv = np.asarray(Wqkv, dtype=np.float32)
    Wo = np.asarray(Wo, dtype=np.float32)
    bo = np.asarray(bo, dtype=np.float32)

    # Wqkv rows per head h: [h*3D, h*3D+D) = q, [+D, +2D) = k, [+2D, +3D) = v.
    # q,k channel order: per pair -> [q(2p)|q(2p+1)], [k(2p)|k(2p+1)] tiles.
    order = []
    for p in range(NPAIR):
        for h in (2 * p, 2 * p + 1):
            order.extend(range(h * 3 * D, h * 3 * D + D))          # q rows
        for h in (2 * p, 2 * p + 1):
            order.extend(range(h * 3 * D + D, h * 3 * D + 2 * D))  # k rows
    wb = np.ascontiguousarray(Wqkv[order].T).astype(ml_dtypes.bfloat16)
    v_order = [h * 3 * D + 2 * D + d for h in range(NH) for d in range(D)]
    # WVS scale lifts Wv out of the fp8e4 subnormal range; the WVS-valued
    # denominator column in vta cancels it exactly in the softmax divide.
    wvq = np.ascontiguousarray(Wqkv[v_order].T * WVS).astype(
        ml_dtypes.float8_e4m3)                                      # (C, C)
    wf = np.ascontiguousarray(np.concatenate(
        [Wo.T, bo[:, None], np.full((C, 1), EXP_BIAS, np.float32)], axis=1
    ))                                                              # (C, C+2)

    out = []
    for b in range(B):
        xc = np.ascontiguousarray(x[b].reshape(C, S))
        out.append(dict(
            xb=xc.astype(ml_dtypes.bfloat16),
            xq=xc.astype(ml_dtypes.float8_e4m3),
            xf=xc, wb=wb, wvq=wvq, wf=wf,
        ))
    return out


_NC_CACHE = []

try:
    # bass_exec HLO does not embed the BIR; bust jax's executable cache so a
    # rebuilt kernel is actually recompiled instead of hitting a stale NEFF.
    import jax as _jax

    _jax.clear_caches()
except Exception:
    pass


def get_nc():
    if not _NC_CACHE:
        _NC_CACHE.append(build_nc())
    return _NC_CACHE[0]


def run(in_maps, **kwargs):
    return run_bass_kernel_spmd(get_nc(), in_maps, core_ids=list(range(B)), **kwargs)


def kernel(x, Wqkv, Wo, bo):
    in_maps = host_inputs(x, Wqkv, Wo, bo)
    r = run(in_maps)
    y = np.stack([r.results[b]["y"].reshape(C, H, W) for b in range(B)])
    return y.astype(np.float32)


if __name__ == "__main__":
    nc = build_nc()
    print("built ok:", len(nc.inst_map), "instructions")


# revision 45
# speedup vs baseline: 1.0004x; 1.0004x over previous
"""Trainium2 Bass kernel for nn_AttentionBlock (B=8, C=512, H=W=32, 8 heads).

Sharding: data-parallel over batch — core b computes batch image b end-to-end
(attention is independent per (batch, head); weights replicated to all cores).

Per-core pipeline (x viewed as (C=512, S=1024), channels-on-partition):
  P1a: q,k = Wqk^T.T @ x (bf16 in, f32 PSUM->SBUF), channel order arranged on
       host so each 128-row m-tile is one head-PAIR of q or k.
  P1b: vT = x.T @ (16*Wv) as fp8 DoubleRow matmuls (2 k-tiles per instr,
       0.5 cycles/row).  Evicted to SBUF fp8 with a 16-valued column per head
       (head stride 66 keeps the DoubleRow weight AP 16B-aligned); the
       denominator row scales by the same 16, so softmax is exact.
  P2 : scoresT[t,s] per (head, j) into a double-buffered (128,1024) PSUM slot;
       emission runs one j-pair AHEAD of P3 so the PE always has the next
       scores ready before the ACT needs them.
  exp: ACT exp(s/8 - 2) PSUM->SBUF fp8e4, one instr per j (the ACT is the
       bottleneck engine at ~68us busy; everything else hides under it).
       The -2 bias (shipped as an extra weights-DMA column) is
       softmax-invariant and keeps exp outputs inside fp8e4 range.
  P3 : attn@v as fp8 DoubleRow matmuls — each instruction contracts BOTH
       j-tiles of a pair (2x128 rows) at 0.5 cycles/row.  Row 64 of the
       65-row output = 16*denominator.
  norm: DVE reciprocal -> ones outer-product PE broadcast -> DVE multiply,
       flushed in two stages one/two units after the head so the PE FIFO
       never parks on a reciprocal-dependent broadcast.
  head 7 runs as two n-half passes: its n0 attention output, normalize and
       P4 n0-half tail (full k-loop matmuls + adds + y DMAs) complete while
       the n1 exps still stream; the endgame holds only the n1 norm chain,
       pair-3 k-instrs (k0-2 pre-accumulated into the freed score banks),
       four adds, and a y DMA fan-out whose ~1.3us launch latencies
       parallelize across the three DGE queues.

PE warmup matmuls run during the initial DMA so the p-state ramp completes
before real work.  Cross-engine waits are kept inside walrus's single-wait
instruction budget with cost-free standalone ldweights carriers (PE), tiny
scratch copies (DVE), and _strip_self_waits.
"""

import os
import sys

for _p in ("/opt/trn_rl_repo", "/root/.axon_site/_ro/trn_rl_repo"):
    if os.path.isdir(_p) and _p not in sys.path:
        sys.path.insert(0, _p)

from contextlib import ExitStack

import ml_dtypes
import numpy as np

import concourse.bass as bass
import concourse.tile as tile
from concourse import mybir
from concourse.bass_utils import run_bass_kernel_spmd

B, C, H, W = 8, 512, 32, 32
NH, D = 8, 64
S = H * W            # 1024 sequence positions
P = 128              # partitions
KT = C // P          # 4 contraction tiles over channels
NT = S // P          # 8 t-tiles
NPAIR = NH // 2      # 4 head pairs
HB = 66              # vta head block: 64 v cols + denom col + pad (16B stride)
F32 = mybir.dt.float32
BF16 = mybir.dt.bfloat16
FP8 = mybir.dt.float8e4
AF = mybir.ActivationFunctionType
ALU = mybir.AluOpType
PM = mybir.MatmulPerfMode

NWARM = int(os.environ.get("K_NWARM", "4"))
EXP_BIAS = -2.0      # exp(s/8 - 2): softmax-invariant, keeps fp8e4 in range
WVS = 16.0           # host scale on Wv; denom column = 16 compensates exactly


def _r(ap):
    """float32r view: fp32 bits at full-rate 1 cycle/row in the PE."""
    return ap.bitcast(mybir.dt.float32r)


def _install_drain_split():
    """walrus's CTRL_NO (drain) codegen accepts only a single semaphore wait,
    but Tile's kernel-tail drain aggregates one wait per live proc.  Split
    them across several serial drains (semantically identical: all complete
    before the closing all-engine barrier)."""
    if getattr(tile.TileContext, "_drain_split_installed", False):
        return
    from concourse.vector_clock import ScopedClock

    orig = tile.TileContext._drain_and_barrier

    def patched(self, tick_clock, wait_clock):
        nc = self.nc
        drain_inst = nc.sync.drain()
        wait_clock.add_sem_waits(
            drain_inst.ins, ScopedClock({None: tick_clock.global_clock})
        )
        si = drain_inst.ins.sync_info
        if si is not None and si.on_wait and len(si.on_wait) > 1:
            waits = list(si.on_wait)
            drain_inst.ins.sync_info = mybir.SyncInfo(
                on_wait=[waits[0]], on_update=list(si.on_update or [])
            )
            for w in waits[1:]:
                d2 = nc.sync.drain()
                d2.ins.sync_info = mybir.SyncInfo(on_wait=[w], on_update=[])

        nc.all_engine_barrier()
        assert self.sems is not None
        popped = nc._tile_sem_poison_stack.pop()
        assert popped is self._sem_poison
        nc.clear_and_free_semaphores(list(self.sems.allocated().values()))
        nc.all_engine_barrier()

    tile.TileContext._drain_and_barrier = patched
    tile.TileContext._drain_split_installed = True
    tile.TileContext._drain_and_barrier_orig = orig


def trace_kernel(ctx, tc, nc, xb, xq_d, xf_d, wb, wvq_d, wf, y):
    cst = ctx.enter_context(tc.tile_pool(name="cst", bufs=1))
    qkp = ctx.enter_context(tc.tile_pool(name="qkp", bufs=5))
    expp = ctx.enter_context(tc.tile_pool(name="expp", bufs=8))
    rdp = ctx.enter_context(tc.tile_pool(name="rdp", bufs=3))
    rbp = ctx.enter_context(tc.tile_pool(name="rbp", bufs=3))
    pa = ctx.enter_context(tc.tile_pool(name="pa", bufs=2, space="PSUM"))
    psc = ctx.enter_context(tc.tile_pool(name="psc", bufs=2, space="PSUM"))
    poa = ctx.enter_context(tc.tile_pool(name="poa", bufs=1, space="PSUM"))

    xt = cst.tile([P, KT, S], BF16)
    xq = cst.tile([P, KT, S], FP8)
    xf = cst.tile([P, KT, S], F32)
    wq = cst.tile([P, KT, 2 * C], BF16)
    wvq = cst.tile([P, KT, C], FP8)
    wo = cst.tile([P, KT, C], F32)
    bob = cst.tile([P, KT, 2], F32)
    onesf = cst.tile([1, D], F32)
    wrm = cst.tile([P, 512], BF16)
    vta = cst.tile([P, NT, NH * HB], FP8)
    res = cst.tile([P, NPAIR, S], F32)
    ybig = cst.tile([P, KT, S], F32)
    scr = cst.tile([1, 256], F32)

    scr_i = [0]

    def dve_sync(*aps):
        # DVE wait-carrier: absorb one cross-engine wait per tiny copy.
        # Disjoint scratch columns avoid WAW self-waits between carriers.
        for ap in aps:
            n = ap.free_size()
            o = (scr_i[0] % 30) * 8
            scr_i[0] += 1
            nc.vector.tensor_copy(scr[0:1, o:o + n], ap)

    def pe_ld(ap):
        # PE wait-carrier: a standalone ldweights is cost-free and absorbs
        # one cross-engine wait into PE program order (f32 views bitcast to
        # bf16 — ldweights rejects 4-byte dtypes).
        if ap.dtype in (F32, mybir.dt.float32r):
            ap = ap.bitcast(BF16)
        nc.tensor.ldweights(ap)

    def pe_ld_dve(ap):
        # pe_ld needs a partition-0-aligned source; for DVE-written regions
        # on other partitions, bounce the tick through a scratch column.
        n = ap.free_size()
        o = (scr_i[0] % 30) * 8
        scr_i[0] += 1
        nc.vector.tensor_copy(scr[0:1, o:o + n], ap)
        pe_ld(scr[0:1, o:o + n])

    # ---- warmup + on-chip consts (no DMA deps) ----
    nc.vector.memset(wrm[:, :], 0.125)
    # denominator column per head in the augmented vT (= WVS, matching the
    # host-side WVS scale on Wv)
    vones = vta.rearrange("p j (h e) -> p j h e", e=HB)[:, :, :, D:D + 1]
    nc.vector.memset(vones, WVS)
    # ones row for the 1/denom broadcast outer product (ACT absorbs the
    # DVE-memset waits here, acting as the ACT-side carrier)
    nc.scalar.activation(_r(onesf[:, :]), wrm[0:1, 0:D], AF.Exp, scale=0.0)
    for i in range(NWARM):
        wt = pa.tile([P, 512], F32, tag="sp", name=f"wrm{i}")
        nc.tensor.matmul(wt[:, :], wrm[:, 0:P], wrm[:, :],
                         start=True, stop=True, skip_group_check=True)

    # ---- DMAs: startup-critical chain on the sync (HWDGE) queue, bulk on
    # the gpsimd queue so the two streams proceed independently ----
    xbr = xb.rearrange("(k p) s -> p k s", p=P)
    xqr = xq_d.rearrange("(k p) s -> p k s", p=P)
    wbr = wb.rearrange("(k p) s -> p k s", p=P)
    wvr = wvq_d.rearrange("(k p) s -> p k s", p=P)
    wfr = wf.rearrange("(k p) s -> p k s", p=P)
    xfr = xf_d.rearrange("(k p) s -> p k s", p=P)
    nc.sync.dma_start(out=xt[:, :, 0:512], in_=xbr[:, :, 0:512])
    nc.sync.dma_start(out=wq[:, :, 0:256], in_=wbr[:, :, 0:256])
    nc.sync.dma_start(out=bob[:, :, :], in_=wfr[:, :, C:C + 2])
    nc.sync.dma_start(out=xt[:, :, 512:S], in_=xbr[:, :, 512:S])
    nc.sync.dma_start(out=xq[:, :, :], in_=xqr[:, :, :])
    nc.sync.dma_start(out=wvq[:, :, :], in_=wvr[:, :, :])
    nc.sync.dma_start(out=wq[:, :, 256:2 * C], in_=wbr[:, :, 256:2 * C])
    nc.sync.dma_start(out=_r(wo[:, :, :]), in_=_r(wfr[:, :, 0:C]))
    nc.sync.dma_start(out=xf[:, :, :], in_=xfr[:, :, :])
    # absorb the bias-column DMA tick into the ACT clock (exps read it)
    nc.scalar.copy(scr[0:1, 252:254], bob[0:1, 0, 0:2])

    qk = [None] * NPAIR

    def p1a_half(m, n):
        pair, isk = divmod(m, 2)
        if isk == 0 and n == 0:
            qk[pair] = qkp.tile([P, 2 * S], F32, tag="qk", name=f"qk{pair}")
        pe_ld(wq[0:1, 0, m * P:m * P + 2])
        pe_ld(xt[0:1, 0, n * 512:n * 512 + 2])
        acc = pa.tile([P, 512], F32, tag="sp", name=f"pacc{m}_{n}")
        for k in range(KT):
            nc.tensor.matmul(
                acc[:, :],
                wq[:, k, m * P:(m + 1) * P],
                xt[:, k, n * 512:(n + 1) * 512],
                start=(k == 0),
                stop=(k == KT - 1),
            )
        nc.vector.tensor_copy(
            _r(qk[pair][:, isk * S + n * 512: isk * S + (n + 1) * 512]),
            _r(acc[:, :]),
        )

    def p1b(j):
        # vT j-tile via fp8 DoubleRow: 2 instrs contract all 4 k-tiles
        pe_ld(wvq[0:1, 0, 0:2])
        pe_ld(xq[0:1, 0, 0:2])
        acc = pa.tile([P, 512], F32, tag="sp", name=f"vacc{j}")
        for kk in range(2):
            nc.tensor.matmul(
                acc[:, :],
                xq[:, 2 * kk:2 * kk + 2, j * P:(j + 1) * P],
                wvq[:, 2 * kk:2 * kk + 2, :],
                start=(kk == 0), stop=(kk == 1),
                perf_mode=PM.DoubleRow, skip_group_check=True,
            )
        nc.vector.tensor_copy(
            vta[:, j, :].rearrange("p (h e) -> p h e", e=HB)[:, :, 0:D],
            acc.rearrange("p (h d) -> p h d", h=NH),
        )

    # filler schedule: PE work slotted into the attention j-pair windows
    fillers = {
        (0, 0): [lambda: p1b(0), lambda: p1b(1), lambda: p1a_half(1, 1),
                 lambda: pe_ld(qk[0][0:1, 2 * S - 2:2 * S])],
        (0, 1): [lambda: p1b(2), lambda: p1b(3)],
        (0, 2): [lambda: p1b(4), lambda: p1b(5)],
        (0, 3): [lambda: p1b(6), lambda: p1b(7)],
    }
    half_order = [(m, n) for pr in (1, 2, 3) for n in (0, 1)
                  for m in (2 * pr, 2 * pr + 1)]
    half_slots = [(1, 0), (1, 0), (1, 1), (1, 2),
                  (1, 3), (2, 0), (2, 1), (2, 2),
                  (2, 3), (3, 0), (3, 1), (3, 2)]
    for (m, n), (h, jp) in zip(half_order, half_slots):
        fillers.setdefault((h, jp), []).append(lambda m=m, n=n: p1a_half(m, n))

    oa_box = [None]

    def norm_recip(h, n):
        oa = oa_box[0]
        # fold the P3 (PE) tick into the DVE clock so the later multiply
        # carries only its rb wait (walrus single-wait + race safety)
        dve_sync(oa[0:1, n * 512:n * 512 + 2])
        rd = rdp.tile([1, 512], F32, tag="rd", name=f"rd{h}_{n}")
        with nc.allow_low_precision(reason="softmax reciprocal"):
            nc.vector.reciprocal(_r(rd[:, :]),
                                 oa[D:D + 1, n * 512:(n + 1) * 512])
        return rd

    def norm_apply(h, n, rd):
        pair, hh = divmod(h, 2)
        oa = oa_box[0]
        rb = rbp.tile([D, 512], F32, tag="rb", name=f"rb{h}_{n}")
        bc = pa.tile([D, 512], F32, tag="sp", name=f"bc{h}_{n}")
        nc.tensor.matmul(
            bc[:, :], _r(onesf[:, :]), _r(rd[0:1, :]),
            start=True, stop=True, skip_group_check=True,
        )
        nc.vector.tensor_copy(rb[:, :], bc[:, :])
        nc.vector.tensor_mul(
            _r(res[D * hh:D * (hh + 1), pair, n * 512:(n + 1) * 512]),
            _r(oa[0:D, n * 512:(n + 1) * 512]), _r(rb[:, :]),
        )

    def norm_half(h, n):
        norm_apply(h, n, norm_recip(h, n))

    ets = {}

    def emit_p2exp(h, jp):
        pair, hh = divmod(h, 2)
        base = D * hh
        qkt = qk[pair]
        if hh == 0 and jp == 0:
            # fresh pair: fold the qk-eviction DVE tick into PE program order
            pe_ld(qkt[0:1, 2 * S - 2:2 * S])
        et = expp.tile([P, 2, S], FP8, tag="et", name=f"et{h}_{jp}")
        ets[(h, jp)] = et
        for jj in range(2):
            j = 2 * jp + jj
            sc = psc.tile([P, S], F32, tag="sc", name=f"sc{h}_{j}")
            for n in range(2):
                nc.tensor.matmul(
                    sc[:, n * 512:(n + 1) * 512],
                    _r(qkt[base:base + D, S + j * P: S + (j + 1) * P]),
                    _r(qkt[base:base + D, n * 512:(n + 1) * 512]),
                    start=True, stop=True, skip_group_check=True,
                )
            nc.scalar.activation(
                et[:, jj, :], sc[:, :], AF.Exp,
                scale=1.0 / np.sqrt(D), bias=bob[:, 0, 1:2],
            )

    def emit_p2exp7(n, jp):
        # head 7 runs as two n-half passes: its n0 attention output, norm and
        # P4 tail complete while the n1 exps still stream on the ACT
        qkt = qk[NPAIR - 1]
        et = expp.tile([P, 2, S], FP8, tag="et", name=f"et7_{n}_{jp}")
        ets[(7, n, jp)] = et
        for jj in range(2):
            j = 2 * jp + jj
            sc = psc.tile([P, S], F32, tag="sc", name=f"sc7_{n}_{j}")
            nc.tensor.matmul(
                sc[:, n * 512:(n + 1) * 512],
                _r(qkt[D:2 * D, S + j * P: S + (j + 1) * P]),
                _r(qkt[D:2 * D, n * 512:(n + 1) * 512]),
                start=True, stop=True, skip_group_check=True,
            )
            nc.scalar.activation(
                et[:, jj, n * 512:(n + 1) * 512],
                sc[:, n * 512:(n + 1) * 512], AF.Exp,
                scale=1.0 / np.sqrt(D), bias=bob[:, 0, 1:2],
            )

    def alloc_oa(h):
        oa_box[0] = poa.tile([D + 1, S], F32, tag="oa", name=f"oa{h}")
        if h > 0:
            # absorb the WAR on oa vs. the previous head's norm reads
            ppair, phh = divmod(h - 1, 2)
            pe_ld_dve(res[D * phh:D * phh + 1, ppair, 0:2])

    def emit_p3(h, jp):
        if jp == 0:
            alloc_oa(h)
            if h <= 1:
                pe_ld(vta[0:1, 2 * jp + 1, 0:2])
        elif h == 0:
            pe_ld(vta[0:1, 2 * jp + 1, 0:2])
        oa = oa_box[0]
        et = ets.pop((h, jp))
        for n in range(2):
            nc.tensor.matmul(
                oa[:, n * 512:(n + 1) * 512],
                vta[:, 2 * jp:2 * jp + 2, h * HB: h * HB + D + 1],
                et[:, :, n * 512:(n + 1) * 512],
                start=(jp == 0), stop=(jp == 3),
                perf_mode=PM.DoubleRow, skip_group_check=True,
            )

    def emit_p3_7(n, jp):
        if n == 0 and jp == 0:
            alloc_oa(7)
        oa = oa_box[0]
        et = ets.pop((7, n, jp))
        nc.tensor.matmul(
            oa[:, n * 512:(n + 1) * 512],
            vta[:, 2 * jp:2 * jp + 2, 7 * HB: 7 * HB + D + 1],
            et[:, :, n * 512:(n + 1) * 512],
            start=(jp == 0), stop=(jp == 3),
            perf_mode=PM.DoubleRow, skip_group_check=True,
        )

    yr = y.rearrange("(k p) s -> p k s", p=P)
    p4acc = {}

    def tail_mm(m, n):
        # P4 m-tile n-half: all four k-instrs (res pair 3 arrives last)
        acc = pa.tile([P, 512], F32, tag="sp", name=f"p4{m}_{n}")
        p4acc[(m, n)] = acc
        for k in range(KT):
            nc.tensor.matmul(
                acc[:, :],
                _r(wo[:, k, m * P:(m + 1) * P]),
                _r(res[:, k, n * 512:(n + 1) * 512]),
                start=(k == 0), stop=(k == KT - 1),
            )

    def tail_add(m, n, q):
        # y = acc + bo + x, then the y half-DMA on an HWDGE queue (SWDGE
        # desc-gen on the Pool engine is ~1.2us per DMA and would serialize
        # the tail)
        nc.vector.scalar_tensor_tensor(
            ybig[:, m, n * 512:(n + 1) * 512], p4acc.pop((m, n))[:, :],
            bob[:, m, 0:1], xf[:, m, n * 512:(n + 1) * 512],
            op0=ALU.add, op1=ALU.add,
        )
        q.dma_start(
            out=yr[:, m, n * 512:(n + 1) * 512],
            in_=ybig[:, m, n * 512:(n + 1) * 512],
        )

    # ---- schedule trace ----
    p1a_half(0, 0)
    p1a_half(1, 0)
    p1a_half(0, 1)
    pend_recip = [None]
    pend_rest = [None]

    def flush_recips():
        # stage 1 (one unit after the head ends): DVE reciprocals only —
        # emitting the PE broadcast here would stall the PE FIFO on them
        if pend_recip[0] is not None:
            h = pend_recip[0]
            pend_recip[0] = None
            pend_rest[0] = (h, norm_recip(h, 0), norm_recip(h, 1))

    def flush_rest():
        # stage 2 (next unit): broadcast + multiply; the recips are long done
        if pend_rest[0] is not None:
            h, rd0, rd1 = pend_rest[0]
            pend_rest[0] = None
            norm_apply(h, 0, rd0)
            norm_apply(h, 1, rd1)

    units = [(h, jp) for h in range(NH - 1) for jp in range(4)]
    emit_p2exp(0, 0)
    for i, (h, jp) in enumerate(units):
        # P2/exp run one j-pair ahead so the ACT never waits on the PE;
        # norms flush in two stages and P3(h,0) defers one unit so the PE
        # FIFO never parks on a reciprocal-dependent broadcast
        if i + 1 < len(units):
            emit_p2exp(*units[i + 1])
        else:
            emit_p2exp7(0, 0)
        flush_rest()
        for f in fillers.get((h, jp), ()):
            f()
        flush_recips()
        if jp == 0 and h > 0:
            pass            # P3(h,0) deferred to the jp==1 unit
        elif jp == 1 and h > 0:
            emit_p3(h, 0)
            emit_p3(h, 1)
        else:
            emit_p3(h, jp)
        if jp == 3:
            pend_recip[0] = h
    # head 7, pass n0
    for jp in range(4):
        if jp < 3:
            emit_p2exp7(0, jp + 1)
        else:
            emit_p2exp7(1, 0)
        flush_rest()
        for f in fillers.get((70, jp), ()):
            f()
        flush_recips()
        if jp == 0:
            pass            # P3_7(0,0) deferred to the jp==1 unit
        elif jp == 1:
            emit_p3_7(0, 0)
            emit_p3_7(0, 1)
        else:
            emit_p3_7(0, jp)
    # head 7, pass n1 — the whole n0-half P4 tail (matmuls, adds, y DMAs)
    # runs inside this pass, overlapped with the n1 exp stream
    rd70 = [None]
    for jp in range(4):
        if jp < 3:
            emit_p2exp7(1, jp + 1)
        if jp == 1:
            rd70[0] = norm_recip(7, 0)
        if jp == 2:
            norm_apply(7, 0, rd70[0])
            dve_sync(xf[0:1, 0, 0:2], bob[0:1, 0, 0:1])
            tail_mm(0, 0)
            tail_mm(1, 0)
        if jp == 3:
            emit_p3_7(1, jp)
            dve_sync(oa_box[0][0:1, 512:514])
            rd7 = rdp.tile([1, 512], F32, tag="rd", name="rd7_1")
            with nc.allow_low_precision(reason="softmax reciprocal"):
                nc.vector.reciprocal(_r(rd7[:, :]),
                                     oa_box[0][D:D + 1, 512:S])
            tail_add(0, 0, nc.sync)
            tail_mm(2, 0)
            tail_add(1, 0, nc.scalar)
            tail_mm(3, 0)
            tail_add(2, 0, nc.gpsimd)
            tail_add(3, 0, nc.sync)
        else:
            emit_p3_7(1, jp)
    # endgame.  The broadcast is consumed straight from PSUM: the multiply
    # reads a staged SBUF copy of oa against the PSUM bc (only both-PSUM is
    # banned), skipping the ACT rb round-trip.  m2/m3 go through the ACT
    # preload(x+bo)+accumulate+evict path so the DVE carries only two final
    # adds; y DMA launch latencies parallelize across the three DGE queues.
    stg = rbp.tile([D, 512], F32, tag="rbt", name="stg7")
    nc.vector.tensor_copy(_r(stg[:, :]), _r(oa_box[0][0:D, 512:S]))
    n1acc = []
    for i in range(2):
        a2 = psc.tile([P, S], F32, tag="sc", name=f"p4n1_{i}")
        n1acc.append(a2)
    for m in (2, 3):
        nc.scalar.activation(
            n1acc[1][:, (m % 2) * 512:(m % 2) * 512 + 512],
            xf[:, m, 512:S], AF.Identity, bias=bob[:, m, 0:1],
        )

    def k012(m):
        acc = n1acc[m // 2][:, (m % 2) * 512:(m % 2) * 512 + 512]
        for k in range(KT - 1):
            nc.tensor.matmul(
                acc, _r(wo[:, k, m * P:(m + 1) * P]),
                _r(res[:, k, 512:S]),
                start=(k == 0 and m < 2), stop=False, skip_group_check=True,
            )

    # the broadcast sits between the two k012 halves so it dispatches as
    # soon as the reciprocal lands instead of behind the whole batch
    k012(0)
    k012(1)
    bc = pa.tile([D, 512], F32, tag="sp", name="bc7_1")
    nc.tensor.matmul(bc[:, :], _r(onesf[:, :]), _r(rd7[0:1, :]),
                     start=True, stop=True, skip_group_check=True)
    k012(2)
    k012(3)
    dve_sync(bc[0:1, 0:2])
    nc.vector.tensor_mul(_r(res[D:2 * D, NPAIR - 1, 512:S]),
                         _r(stg[:, :]), _r(bc[:, :]))
    yq = [nc.sync, nc.scalar, nc.gpsimd, nc.sync]
    for m in range(KT):
        acc = n1acc[m // 2][:, (m % 2) * 512:(m % 2) * 512 + 512]
        nc.tensor.matmul(
            acc, _r(wo[:, KT - 1, m * P:(m + 1) * P]),
            _r(res[:, KT - 1, 512:S]),
            start=False, stop=True, skip_group_check=True,
        )
        if m < 2:
            nc.vector.scalar_tensor_tensor(
                ybig[:, m, 512:S], acc, bob[:, m, 0:1], xf[:, m, 512:S],
                op0=ALU.add, op1=ALU.add,
            )
        else:
            nc.scalar.copy(ybig[:, m, 512:S], acc)
        yq[m].dma_start(out=yr[:, m, 512:S], in_=ybig[:, m, 512:S])


ENGINE_SEM_PREFIX = {
    "PE": "PE_",
    "Activation": "Activation_",
    "DVE": "DVE_",
    "Pool": "Pool_",
    "SP": "SP_",
}


def _strip_self_waits(nc):
    """Drop same-engine semaphore self-waits from multi-wait instructions.

    Engines execute and complete their own instructions in program order
    (PE matmuls are pc-monotone in start and end; ACT/DVE/Pool are strict
    FIFO with per-op drains), so a wait on the engine's own completion
    semaphore is redundant whenever the instruction carries another wait —
    and walrus's PE/ACT instruction structs only encode a single wait.
    """
    n = 0
    for inst in nc.inst_map.values():
        si = getattr(inst, "sync_info", None)
        if si is None or not si.on_wait or len(si.on_wait) <= 1:
            continue
        eng = str(getattr(inst, "engine", "")).split(".")[-1]
        if type(inst).__name__ == "InstDMACopy":
            keep = [w for w in si.on_wait if not w.ant_name.startswith("DMAHW")]
            if keep and len(keep) != len(si.on_wait):
                inst.sync_info = mybir.SyncInfo(
                    on_wait=keep, on_update=list(si.on_update or [])
                )
                n += 1
            continue
        pref = ENGINE_SEM_PREFIX.get(eng)
        if pref is None:
            continue
        keep = [w for w in si.on_wait if not w.ant_name.startswith(pref)]
        if len(keep) != len(si.on_wait) and keep:
            inst.sync_info = mybir.SyncInfo(
                on_wait=keep, on_update=list(si.on_update or [])
            )
            n += 1
    return n


def build_nc():
    _install_drain_split()
    nc = bass.Bass(trn_type="TRN2", debug=False, num_devices=8)

    xb_d = nc.dram_tensor("xb", [C, S], BF16, kind="ExternalInput")
    xq_d = nc.dram_tensor("xq", [C, S], FP8, kind="ExternalInput")
    xf_d = nc.dram_tensor("xf", [C, S], F32, kind="ExternalInput")
    wb_d = nc.dram_tensor("wb", [C, 2 * C], BF16, kind="ExternalInput")
    wvq_d = nc.dram_tensor("wvq", [C, C], FP8, kind="ExternalInput")
    wf_d = nc.dram_tensor("wf", [C, C + 2], F32, kind="ExternalInput")
    y_d = nc.dram_tensor("y", [C, S], F32, kind="ExternalOutput")
    with tile.TileContext(nc) as tc, ExitStack() as ctx:
        trace_kernel(ctx, tc, nc, xb_d.ap(), xq_d.ap(), xf_d.ap(), wb_d.ap(),
                     wvq_d.ap(), wf_d.ap(), y_d.ap())
    _strip_self_waits(nc)
    if not nc.is_finalized():
        nc.finalize()
    return nc


def host_inputs(x, Wqkv, Wo, bo):
    """Host-side reshard: per-core input dicts (weights replicated)."""
    x = np.ascontiguousarray(np.asarray(x, dtype=np.float32))
    Wqkv = np.asarray(Wqkv, dtype=np.float32)
    Wo = np.asarray(Wo, dtype=np.float32)
    bo = np.asarray(bo, dtype=np.float32)

    # Wqkv rows per head h: [h*3D, h*3D+D) = q, [+D, +2D) = k, [+2D, +3D) = v.
    # q,k channel order: per pair -> [q(2p)|q(2p+1)], [k(2p)|k(2p+1)] tiles.
    order = []
    for p in range(NPAIR):
        for h in (2 * p, 2 * p + 1):
            order.extend(range(h * 3 * D, h * 3 * D + D))          # q rows
        for h in (2 * p, 2 * p + 1):
            order.extend(range(h * 3 * D + D, h * 3 * D + 2 * D))  # k rows
    wb = np.ascontiguousarray(Wqkv[order].T).astype(ml_dtypes.bfloat16)
    v_order = [h * 3 * D + 2 * D + d for h in range(NH) for d in range(D)]
    # WVS scale lifts Wv out of the fp8e4 subnormal range; the WVS-valued
    # denominator column in vta cancels it exactly in the softmax divide.
    wvq = np.ascontiguousarray(Wqkv[v_order].T * WVS).astype(
        ml_dtypes.float8_e4m3)                                      # (C, C)
    wf = np.ascontiguousarray(np.concatenate(
        [Wo.T, bo[:, None], np.full((C, 1), EXP_BIAS, np.float32)], axis=1
    ))                                                              # (C, C+2)

    out = []
    for b in range(B):
        xc = np.ascontiguousarray(x[b].reshape(C, S))
        out.append(dict(
            xb=xc.astype(ml_dtypes.bfloat16),
            xq=xc.astype(ml_dtypes.float8_e4m3),
            xf=xc, wb=wb, wvq=wvq, wf=wf,
        ))
    return out


_NC_CACHE = []

try:
    # bass_exec HLO does not embed the BIR; bust jax's executable cache so a
    # rebuilt kernel is actually recompiled instead of hitting a stale NEFF.
    import jax as _jax

    _jax.clear_caches()
except Exception:
    pass


def get_nc():
    if not _NC_CACHE:
        _NC_CACHE.append(build_nc())
    return _NC_CACHE[0]


def run(in_maps, **kwargs):
    return run_bass_kernel_spmd(get_nc(), in_maps, core_ids=list(range(B)), **kwargs)


def kernel(x, Wqkv, Wo, bo):
    in_maps = host_inputs(x, Wqkv, Wo, bo)
    r = run(in_maps)
    y = np.stack([r.results[b]["y"].reshape(C, H, W) for b in range(B)])
    return y.astype(np.float32)


if __name__ == "__main__":
    nc = build_nc()
    print("built ok:", len(nc.inst_map), "instructions")
